# revision 17
# baseline (speedup 1.0000x reference)
"""Bahdanau additive attention on 8 Trainium2 NeuronCores (pure data parallel).

Math (per batch column b):
  key[s,e]   = sum_d value[s,b,d] * Wv[e,d]
  score[s]   = sum_e nv[e] * tanh(pq[b,e] + key[s,e] + bias[e])   (+ additive mask)
  p          = softmax_s(score)
  context[d] = sum_s p[s] * value[s,b,d]

Sharding: batch dim (32) split 4-per-core across 8 cores; weights replicated.

Device layout choices:
  * value shipped pre-transposed as VT[d, b, s] (fp16) so the key projection
    contracts d on partitions:  key_psum[e_blk, s_chunk] = WvT_tile.T @ VT_tile.
  * tanh runs on ScalarE reading PSUM, with (pq+bias) folded in as the
    per-partition activation bias (e on partitions).
  * score matvec contracts e on partitions with a replicated-nv stationary
    operand, so the PSUM result [128, s] carries the scores broadcast across
    all 128 partitions; softmax then needs no cross-partition work at all and
    exp() output doubles as the broadcast weights for the context reduction.
  * context = sum_s p * VT done as fused multiply+reduce on VectorE.
"""

import sys

sys.path.insert(0, "/opt/trn_rl_repo")
import numpy as np

import concourse.bacc as bacc
import concourse.tile as tile
from concourse import mybir
from concourse.bass_utils import run_bass_kernel_spmd

N_CORES = 8
S = 2048
B = 32
E = 1024  # embed dim (= query/value dim here)
BL = B // N_CORES  # local batch per core = 4
NEB = E // 128  # e blocks = 8
NDB = E // 128  # d blocks = 8
NSC = S // 1024  # s chunks of 1024 = 2

F16 = mybir.dt.float16
F32 = mybir.dt.float32
AF = mybir.ActivationFunctionType
ALU = mybir.AluOpType
AX = mybir.AxisListType

_STATE = {}


def _build():
    nc = bacc.Bacc("TRN2", target_bir_lowering=False, debug=False,
                   num_devices=N_CORES)

    vt_d = nc.dram_tensor("vt", [E, BL, S], F16, kind="ExternalInput")
    wvt_d = nc.dram_tensor("wvt", [E, E], F16, kind="ExternalInput")
    nvrep_d = nc.dram_tensor("nvrep", [NEB, 128, 128], F16, kind="ExternalInput")
    pqbt_d = nc.dram_tensor("pqbt", [NEB, 128, BL], F32, kind="ExternalInput")
    mask_d = nc.dram_tensor("maskadd", [BL, S], F16, kind="ExternalInput")
    ones_d = nc.dram_tensor("ones", [1, 128], F16, kind="ExternalInput")
    attn_d = nc.dram_tensor("attn", [BL, S], F32, kind="ExternalOutput")
    ctxt_d = nc.dram_tensor("ctxt", [128, BL * NDB], F32, kind="ExternalOutput")

    with tile.TileContext(nc) as tc:
        with (
            tc.tile_pool(name="const", bufs=1) as const,
            tc.tile_pool(name="vtp", bufs=1) as vtp,
            tc.tile_pool(name="thp", bufs=3) as thp,
            tc.tile_pool(name="ep", bufs=1) as ep,
            tc.tile_pool(name="scrp", bufs=1) as scrp,
            tc.tile_pool(name="stat", bufs=2) as stat,
            tc.tile_pool(name="maskp", bufs=2) as maskp,
            tc.tile_pool(name="ptp", bufs=1) as ptp,
            tc.tile_pool(name="kpp", bufs=2, space="PSUM") as kpp,
            tc.tile_pool(name="spp", bufs=1, space="PSUM") as spp,
        ):
            # ---- constants / weights ----
            wvt_sb = []
            for db in range(NDB):
                w = const.tile([128, E], F16, tag=f"wvt{db}")
                nc.sync.dma_start(w[:], wvt_d.ap()[db * 128:(db + 1) * 128, :])
                wvt_sb.append(w)
            nvrep_sb = const.tile([128, NEB, 128], F16, tag="nvrep")
            nc.sync.dma_start(nvrep_sb[:], nvrep_d.ap().rearrange("a k m -> k a m"))
            pqbt_sb = const.tile([128, NEB, BL], F32, tag="pqbt")
            nc.sync.dma_start(pqbt_sb[:], pqbt_d.ap().rearrange("a k b -> k a b"))
            ones_sb = const.tile([1, 128], F16, tag="ones")
            nc.sync.dma_start(ones_sb[:], ones_d.ap())

            ctxu = const.tile([128, BL * NDB], F32, tag="ctxu")
            rbc = const.tile([128, BL * NDB], F32, tag="rbc")
            ctxf = const.tile([128, BL * NDB], F32, tag="ctxf")

            # ---- value^T tiles, in consumption order ----
            vt_sb = [[None] * BL for _ in range(NDB)]
            for b in range(BL):
                for db in range(NDB):
                    t = vtp.tile([128, S], F16, tag=f"vt{db}_{b}")
                    nc.sync.dma_start(t[:], vt_d.ap()[db * 128:(db + 1) * 128, b, :])
                    vt_sb[db][b] = t

            # ---- main loop ----
            for b in range(BL):
                sp = spp.tile([128, S], F32)  # broadcast scores, 4 banks
                mask_sb = maskp.tile([1, S], F16, tag="mask")
                nc.sync.dma_start(mask_sb[:], mask_d.ap()[b:b + 1, :])
                for sc in range(NSC):
                    for eb in range(NEB):
                        kp = kpp.tile([128, 1024], F32)  # 2 banks
                        for h in range(2):
                            c0 = sc * 1024 + h * 512
                            for db in range(NDB):
                                nc.tensor.matmul(
                                    kp[:, h * 512:(h + 1) * 512],
                                    wvt_sb[db][:, eb * 128:(eb + 1) * 128],
                                    vt_sb[db][b][:, c0:c0 + 512],
                                    start=(db == 0), stop=(db == NDB - 1),
                                )
                        th = thp.tile([128, 1024], F16)
                        nc.scalar.activation(th[:], kp[:], AF.Tanh,
                                             bias=pqbt_sb[:, eb, b:b + 1])
                        for h in range(2):
                            c0 = sc * 1024 + h * 512
                            nc.tensor.matmul(
                                sp[:, c0:c0 + 512],
                                nvrep_sb[:, eb, :],
                                th[:, h * 512:(h + 1) * 512],
                                start=(eb == 0), stop=False,
                            )
                    for h in range(2):
                        c0 = sc * 1024 + h * 512
                        nc.tensor.matmul(
                            sp[:, c0:c0 + 512],
                            ones_sb[0:1, :],
                            mask_sb[0:1, c0:c0 + 512],
                            start=False, stop=True,
                        )

                # softmax over s (rows already broadcast across partitions)
                negm = stat.tile([128, 1], F32, tag="negm")
                nc.vector.reduce_max(negm[:], sp[:], axis=AX.X, negate=True)
                eB = ep.tile([128, S], F32)
                den = stat.tile([128, 1], F32, tag="den")
                nc.scalar.activation(eB[:], sp[:], AF.Exp, bias=negm[:],
                                     accum_out=den[:])
                r = stat.tile([128, 1], F32, tag="r")
                nc.vector.reciprocal(r[:], den[:])
                # attention-probability output row for this b (scores are
                # replicated across partitions, so partition 0 suffices)
                ptmp = ptp.tile([1, S], F32, tag="ptmp")
                nc.vector.tensor_scalar_mul(ptmp[0:1, :], eB[0:1, :],
                                            r[0:1, 0:1])
                nc.sync.dma_start(attn_d.ap()[b:b + 1, :], ptmp[0:1, :])
                # stash 1/den for the final context scaling
                nc.vector.tensor_copy(rbc[:, b * NDB:(b + 1) * NDB],
                                      r[:, 0:1].broadcast_to((128, NDB)))
                # unnormalized context: sum_s exp(score) * value
                # (tensor_tensor_reduce exec-faults on this runtime, so use
                #  a separate multiply + free-dim reduce)
                for db in range(NDB):
                    scr = scrp.tile([128, S], F32)
                    nc.vector.tensor_mul(scr[:], vt_sb[db][b][:], eB[:])
                    nc.vector.reduce_sum(
                        ctxu[:, b * NDB + db:b * NDB + db + 1], scr[:],
                        axis=AX.X)

            nc.vector.tensor_mul(ctxf[:], ctxu[:], rbc[:])
            nc.sync.dma_start(ctxt_d.ap(), ctxf[:])

    nc.compile()
    return nc


def _prep_inputs(query, value, Wq, Wv, v, b, g, key_padding_mask):
    query = np.asarray(query, dtype=np.float32)
    value = np.asarray(value, dtype=np.float32)
    Wq = np.asarray(Wq, dtype=np.float32)
    Wv = np.asarray(Wv, dtype=np.float32)
    v = np.asarray(v, dtype=np.float64)
    bias = np.asarray(b, dtype=np.float64)
    g = np.asarray(g, dtype=np.float64)
    mask = np.asarray(key_padding_mask)

    # small host-side math: query projection and normalized v
    pqb = (query.astype(np.float64) @ Wq.T.astype(np.float64)
           + bias[None, :]).astype(np.float32)  # [B, E]
    nv = (g[0] * v / np.linalg.norm(v)).astype(np.float16)  # [E]

    wvt = np.ascontiguousarray(Wv.T).astype(np.float16)  # [d, e]
    nvrep = np.ascontiguousarray(
        np.broadcast_to(nv.reshape(NEB, 128, 1), (NEB, 128, 128)))
    ones = np.ones((1, 128), dtype=np.float16)

    v16 = value.astype(np.float16)  # [S, B, E]
    maskT = mask.T  # [B, S]

    in_maps = []
    for c in range(N_CORES):
        sl = slice(BL * c, BL * (c + 1))
        vt = np.ascontiguousarray(v16[:, sl, :].transpose(2, 1, 0))  # [E, BL, S]
        pqbt = np.ascontiguousarray(pqb[sl, :].T).reshape(NEB, 128, BL)
        maskadd = np.where(maskT[sl, :], np.float16(-60000.0),
                           np.float16(0.0)).astype(np.float16)
        in_maps.append({
            "vt": vt, "wvt": wvt, "nvrep": nvrep,
            "pqbt": pqbt.astype(np.float32), "maskadd": maskadd, "ones": ones,
        })
    return in_maps


def _unshard(results):
    attn_rows = []
    ctx_rows = []
    for c in range(N_CORES):
        attn_rows.append(np.asarray(results[c]["attn"], dtype=np.float32))
        ct = np.asarray(results[c]["ctxt"], dtype=np.float32)  # [128, BL*NDB]
        ct = ct.reshape(128, BL, NDB).transpose(1, 2, 0).reshape(BL, E)
        ctx_rows.append(ct)
    attn = np.concatenate(attn_rows, axis=0).T.copy()  # [S, B]
    context = np.concatenate(ctx_rows, axis=0)  # [B, E]
    return context, attn


def kernel(query, value, Wq, Wv, v, b, g, key_padding_mask):
    if "nc" not in _STATE:
        _STATE["nc"] = _build()
    nc = _STATE["nc"]
    in_maps = _prep_inputs(query, value, Wq, Wv, v, b, g, key_padding_mask)
    res = run_bass_kernel_spmd(nc, in_maps, list(range(N_CORES)))
    context, attn = _unshard(res.results)
    return context, attn, attn


def bench(in_maps, iters=10):
    """Repeatedly execute the compiled NEFF via PJRT; return per-run wall
    times (s). First entry includes compile/trace overhead."""
    import time

    import jax
    import jax.numpy as jnp
    from jax.sharding import Mesh, PartitionSpec
    from jax.experimental.shard_map import shard_map

    from concourse import bass2jax
    from concourse.bass2jax import _bass_exec_p, install_neuronx_cc_hook
    from concourse import mybir as mb

    if "nc" not in _STATE:
        _STATE["nc"] = _build()
    nc = _STATE["nc"]
    install_neuronx_cc_hook()

    partition_name = (nc.partition_id_tensor.name
                      if nc.partition_id_tensor else None)
    in_names, out_names, out_avals, zero_outs = [], [], [], []
    for alloc in nc.m.functions[0].allocations:
        if not isinstance(alloc, mb.MemoryLocationSet):
            continue
        name = alloc.memorylocations[0].name
        if alloc.kind == "ExternalInput":
            if name != partition_name:
                in_names.append(name)
        elif alloc.kind == "ExternalOutput":
            out_names.append(name)
            shape = tuple(alloc.tensor_shape)
            dtype = mb.dt.np(alloc.dtype)
            out_avals.append(jax.core.ShapedArray(shape, dtype))
            zero_outs.append(np.zeros(shape, dtype))
    n_params = len(in_names)
    all_in = in_names + out_names + ([partition_name] if partition_name else [])

    def _body(*args):
        operands = list(args)
        if partition_name is not None:
            operands.append(bass2jax.partition_id_tensor())
        return tuple(_bass_exec_p.bind(
            *operands, out_avals=tuple(out_avals), in_names=tuple(all_in),
            out_names=tuple(out_names), lowering_input_output_aliases=(),
            sim_require_finite=False, sim_require_nnan=False, nc=nc))

    devices = jax.devices()[:N_CORES]
    mesh = Mesh(np.asarray(devices), ("core",))
    n_outs = len(out_names)
    fn = jax.jit(shard_map(_body, mesh=mesh,
                           in_specs=(PartitionSpec("core"),) * (n_params + n_outs),
                           out_specs=(PartitionSpec("core"),) * n_outs,
                           check_rep=False), keep_unused=True)

    concat_in = [np.concatenate([np.asarray(in_maps[c][nm])
                                 for c in range(N_CORES)], axis=0)
                 for nm in in_names]
    concat_zero = [np.concatenate([z] * N_CORES, axis=0) for z in zero_outs]
    dev_in = [jax.device_put(a) for a in concat_in]
    dev_zero = [jax.device_put(a) for a in concat_zero]

    times = []
    for _ in range(iters):
        t0 = time.perf_counter()
        outs = fn(*dev_in, *dev_zero)
        jax.block_until_ready(outs)
        times.append(time.perf_counter() - t0)
    return times
